# revision 4
# baseline (speedup 1.0000x reference)
"""Trainium2 Bass kernel for MixerDiffAttention (differential attention).

Sharding: tensor-parallel over the 8 (n_head//2) head groups across 8 cores
(data-parallel over B is trivial since B=1). Each core computes the QKV
projections for its head group, both differential attention branches, the
normalized combination y1 - lambda*y2, and its head's partial product with
the row-sharded c_proj. The host sums the 8 partial outputs (the unshard
step for row-parallel tensor parallelism).

v2 layout notes (per core, head h):
  - QKV projections x-stationary: out[t, (q1 q2 k1 k2 v)] per 128-row t-block,
    accumulated over 8 contraction chunks.
  - Rotary is applied to RAW q/k, then the rms_norm scale is applied to the
    rotated values (the rotation is orthogonal per (i, i+32) pair, so it
    commutes with the per-subhead scalar normalization).
  - 1/sqrt(msq+eps) is computed as exp(-0.5*ln(msq+eps)) so the only ACT
    table set used in the whole kernel is natural_log_exp (no table thrash
    between the norm and the attention exps).
  - q/k blocks are transposed to [c, t] with the DMA xbar (idle engine)
    instead of PE transposes + ACT evacuations.
  - scores for the two branches are written into one 2-bank PSUM tile and
    exp'd in a single ACT op; the two score matmuls have K=64 at partition
    bases 0/64 so they run concurrently in the PE array (row tiling).
  - exp never overflows: q/k are rms-normalized so |score*scale| <= 8.
  - stage B (QKV+rotary per 4-t-block group) and the causal attention chunks
    are interleaved in emission order to keep the PE stream dense (HAM warm).
"""

import os
import sys

import numpy as np

for _p in ("/opt/trn_rl_repo", "/root/.axon_site/_ro/trn_rl_repo"):
    if os.path.isdir(_p) and _p not in sys.path:
        sys.path.insert(0, _p)

import ml_dtypes

import concourse.bass as bass
import concourse.mybir as mybir
import concourse.tile as tile
from concourse import bacc
from concourse.bass import ds, ts
from concourse.bass_utils import run_bass_kernel_spmd

BF16 = mybir.dt.bfloat16
F32 = mybir.dt.float32
AF = mybir.ActivationFunctionType
ALU = mybir.AluOpType

N_HEAD = 16
D = 1024
HD = 64  # head dim
T = 2048
NCORES = 8
TB = T // 128  # 16 t-blocks
KC = D // 128  # 8 contraction chunks
NTC = T // 512  # 4 t-chunks of 512
LAMBDA_INIT = 0.8 - 0.6 * float(np.exp(-0.3 * 1))
EPS = float(np.finfo(np.float32).eps)
SCALE = 1.0 / 8.0  # 1/sqrt(64)

_CACHE = {}


def _build_program(lam: float) -> bass.Bass:
    nc = bacc.Bacc("TRN2", target_bir_lowering=False, debug=False)

    xT = nc.declare_dram_parameter("xT", [D, T], BF16, isOutput=False)
    wqkv = nc.declare_dram_parameter("wqkv", [D, 384], BF16, isOutput=False)
    wpp = nc.declare_dram_parameter("wpp", [128, D], BF16, isOutput=False)
    cos_d = nc.declare_dram_parameter("cos", [128, TB * 32], BF16, isOutput=False)
    sin_d = nc.declare_dram_parameter("sin", [128, TB * 32], BF16, isOutput=False)
    diag_d = nc.declare_dram_parameter("diag", [128, 128], BF16, isOutput=False)
    outTp = nc.declare_dram_parameter("outTp", [D, T], BF16, isOutput=True)

    with tile.TileContext(nc) as tc:
        with (
            tc.tile_pool(name="const", bufs=1) as cpool,
            tc.tile_pool(name="work", bufs=3) as wpool,
            tc.tile_pool(name="ptile", bufs=3) as ppool,
            tc.tile_pool(name="ostage", bufs=3) as opool,
            tc.tile_pool(name="psum", bufs=1, space="PSUM") as psum_pool,
        ):
            # ---- persistent SBUF tensors ----
            xT_sb = cpool.tile([128, KC, T], BF16, tag="xT")
            wqkv_sb = cpool.tile([128, KC, 384], BF16, tag="wqkv")
            wpp_sb = cpool.tile([128, KC, 128], BF16, tag="wpp")
            cos_sb = cpool.tile([128, TB, 32], BF16, tag="cos")
            sin_sb = cpool.tile([128, TB, 32], BF16, tag="sin")
            diag_sb = cpool.tile([128, 128], BF16, tag="diag")
            ones_sb = cpool.tile([128, 128], BF16, tag="ones")
            qk_sb = cpool.tile([128, TB, 4, HD], BF16, tag="qk")  # raw q1 q2 k1 k2
            ssq_sb = cpool.tile([128, TB, 4], F32, tag="ssq")
            rsc_sb = cpool.tile([128, TB, 4], F32, tag="rsc")
            qT_sb = cpool.tile([128, T], BF16, tag="qT")  # rows 0:64 g0, 64:128 g1
            kT_sb = cpool.tile([128, T], BF16, tag="kT")
            v_sb = cpool.tile([128, TB, 128], BF16, tag="v")  # [s-part, tb, j]

            # ---- load constants ----
            # wqkv (sync) + x t-chunk 0 (gpsimd/scalar) gate the first QKV
            # matmuls; everything else streams in behind them.
            for kc in range(KC):
                nc.sync.dma_start(out=wqkv_sb[:, kc, :], in_=wqkv[ts(kc, 128), :])
            for kc in range(KC):
                eng = nc.gpsimd if kc % 2 == 0 else nc.scalar
                eng.dma_start(
                    out=xT_sb[:, kc, ts(0, 512)], in_=xT[ts(kc, 128), ts(0, 512)]
                )
            nc.scalar.dma_start(
                out=cos_sb[:].rearrange("p a b -> p (a b)"), in_=cos_d[:, :]
            )
            nc.scalar.dma_start(
                out=sin_sb[:].rearrange("p a b -> p (a b)"), in_=sin_d[:, :]
            )
            nc.gpsimd.dma_start(out=diag_sb[:], in_=diag_d[:, :])
            for tc_i in range(1, NTC):
                for kc in range(KC):
                    eng = (nc.sync, nc.gpsimd, nc.scalar)[kc % 3]
                    eng.dma_start(
                        out=xT_sb[:, kc, ts(tc_i, 512)],
                        in_=xT[ts(kc, 128), ts(tc_i, 512)],
                    )
            for kc in range(KC):
                nc.gpsimd.dma_start(out=wpp_sb[:, kc, :], in_=wpp[:, ts(kc, 128)])
            nc.vector.memset(ones_sb[:], 1.0)
            eps_sb = cpool.tile([128, 1], F32, tag="eps")
            nc.vector.memset(eps_sb[:], EPS)

            # ---- stage B: QKV projection for one group of 4 t-blocks,
            # then rotary + rms scale + DMA transposes for the group ----
            def stage_b(c):
                for i in range(4):
                    tb = 4 * c + i
                    pqkv = psum_pool.tile(
                        [128, 2, 512], F32, tag="pp", bufs=2, name="pqkv"
                    )
                    qkv = pqkv[:].rearrange("p a b -> p (a b)")
                    for kc in range(KC):
                        nc.tensor.matmul(
                            qkv[:, 0:384],
                            xT_sb[:, kc, ts(tb, 128)],
                            wqkv_sb[:, kc, :],
                            start=(kc == 0),
                            stop=(kc == KC - 1),
                        )
                    # evacuate: v (ACT copy), raw qk (DVE cast)
                    nc.scalar.copy(v_sb[:, tb, :], qkv[:, 256:384])
                    nc.vector.tensor_copy(
                        qk_sb[:, tb].rearrange("p a b -> p (a b)"), qkv[:, 0:256]
                    )

                # sum of squares per 64-wide subhead, for the whole group
                sq = wpool.tile([128, 4, 4, HD], BF16, tag="sq")
                qkg = qk_sb[:, ds(4 * c, 4)]
                nc.vector.tensor_mul(sq[:], qkg, qkg)
                nc.vector.reduce_sum(
                    ssq_sb[:, ds(4 * c, 4)], sq[:], axis=mybir.AxisListType.X
                )
                # rsc = 1/sqrt(ssq/64 + eps) = exp(-0.5 * ln(ssq/64 + eps));
                # ln+exp live in one ACT table set, so the whole kernel uses a
                # single set (no table reloads between norm and attention).
                lssq = wpool.tile([128, 4, 4], F32, tag="lssq")
                nc.scalar.activation(
                    lssq[:], ssq_sb[:, ds(4 * c, 4)], AF.Ln, bias=eps_sb[:], scale=1.0 / HD
                )
                nc.scalar.activation(
                    rsc_sb[:, ds(4 * c, 4)], lssq[:], AF.Exp, scale=-0.5
                )

                # rotary on raw qk: out1 = n1*cos + n2*sin ; out2 = n2*cos - n1*sin
                n1 = qkg[:, :, :, 0:32]
                n2 = qkg[:, :, :, 32:64]
                cosb = (
                    cos_sb[:, ds(4 * c, 4), :].unsqueeze(2).broadcast_to([128, 4, 4, 32])
                )
                sinb = (
                    sin_sb[:, ds(4 * c, 4), :].unsqueeze(2).broadcast_to([128, 4, 4, 32])
                )
                rot = wpool.tile([128, 4, 4, HD], BF16, tag="rot")
                t1 = wpool.tile([128, 4, 4, 32], BF16, tag="t1")
                t2 = wpool.tile([128, 4, 4, 32], BF16, tag="t2")
                nc.vector.tensor_mul(t1[:], n1, cosb)
                nc.vector.tensor_mul(t2[:], n2, sinb)
                nc.vector.tensor_add(rot[:, :, :, 0:32], t1[:], t2[:])
                nc.vector.tensor_mul(t1[:], n2, cosb)
                nc.vector.tensor_mul(t2[:], n1, sinb)
                nc.vector.tensor_sub(rot[:, :, :, 32:64], t1[:], t2[:])

                # apply the rms_norm scale to the rotated values
                qkr = wpool.tile([128, 4, 4, HD], BF16, tag="qkr")
                rscb = (
                    rsc_sb[:, ds(4 * c, 4)].unsqueeze(3).broadcast_to([128, 4, 4, HD])
                )
                nc.vector.tensor_mul(qkr[:], rot[:], rscb)

                # DMA xbar transposes: [t, c]-block -> [c, t]-block
                for i in range(4):
                    tb = 4 * c + i
                    qblk = qkr[:, i, 0:2, :].rearrange("p a b -> p (a b)")
                    kblk = qkr[:, i, 2:4, :].rearrange("p a b -> p (a b)")
                    nc.sync.dma_start(
                        out=qT_sb[:, ts(tb, 128)], in_=qblk, transpose=True
                    )
                    nc.scalar.dma_start(
                        out=kT_sb[:, ts(tb, 128)], in_=kblk, transpose=True
                    )

            # ---- stage C: differential attention + partial projection for
            # one 512-wide t-chunk ----
            def stage_c(tc_i):
                nsb = 4 * tc_i + 4  # s-blocks touching this t-chunk
                py = psum_pool.tile([128, 2, 512], F32, tag="py", name="py")
                pd = psum_pool.tile([128, 2, 512], F32, tag="pd", name="pd")
                for si in range(nsb):
                    col0 = max(0, si * 128 - tc_i * 512)
                    w = 512 - col0
                    pp = psum_pool.tile(
                        [128, 2, 512], F32, tag="pp", bufs=2, name="pp"
                    )
                    for g in range(2):
                        nc.tensor.matmul(
                            pp[:, g, col0:512],
                            kT_sb[ds(g * 64, 64), ts(si, 128)],
                            qT_sb[ds(g * 64, 64), ds(tc_i * 512 + col0, w)],
                            start=True,
                            stop=True,
                        )
                    pt = ppool.tile([128, 2, 512], BF16, tag="pt")
                    nc.scalar.activation(
                        pt[:, :, col0:512], pp[:, :, col0:512], AF.Exp, scale=SCALE
                    )
                    if col0 > 0 or si * 128 == tc_i * 512:
                        # diagonal block: zero out s > t inside it
                        diagb = diag_sb[:].unsqueeze(1).broadcast_to([128, 2, 128])
                        nc.vector.tensor_mul(
                            pt[:, :, col0 : col0 + 128],
                            pt[:, :, col0 : col0 + 128],
                            diagb,
                        )
                    for g in range(2):
                        nc.tensor.matmul(
                            py[:, g, col0:512],
                            v_sb[:, si, :],
                            pt[:, g, col0:512],
                            start=(si == 0),
                            stop=(si == nsb - 1),
                        )
                    for g in range(2):
                        nc.tensor.matmul(
                            pd[:, g, col0:512],
                            ones_sb[:],
                            pt[:, g, col0:512],
                            start=(si == 0),
                            stop=(si == nsb - 1),
                        )

                # normalize + combine the two branches
                rec = wpool.tile([128, 2, 512], F32, tag="rec")
                nc.vector.reciprocal_approx_fast(rec[:], pd[:])
                yn = wpool.tile([128, 2, 512], BF16, tag="yn")
                nc.vector.tensor_mul(yn[:], py[:], rec[:])
                ycomb = wpool.tile([128, 512], BF16, tag="ycomb")
                nc.vector.scalar_tensor_tensor(
                    ycomb[:], yn[:, 1], -lam, yn[:, 0], ALU.mult, ALU.add
                )

                # partial projection for this t-chunk
                for icp in range(KC // 2):
                    po = psum_pool.tile([128, 2, 512], F32, tag="pp", bufs=2, name="po")
                    for j in range(2):
                        ic = icp * 2 + j
                        nc.tensor.matmul(
                            po[:, j, :],
                            wpp_sb[:, ic, :],
                            ycomb[:],
                            start=True,
                            stop=True,
                        )
                    ost = opool.tile([128, 2, 512], BF16, tag="ost")
                    eng = nc.vector if icp % 2 == 0 else nc.scalar
                    if icp % 2 == 0:
                        nc.vector.tensor_copy(ost[:], po[:])
                    else:
                        nc.scalar.copy(ost[:], po[:])
                    for j in range(2):
                        ic = icp * 2 + j
                        nc.sync.dma_start(
                            out=outTp[ts(ic, 128), ts(tc_i, 512)], in_=ost[:, j]
                        )

            # ---- emission schedule: interleave stage B groups with the
            # attention chunks so the PE matmul stream stays dense ----
            stage_b(0)
            stage_b(1)
            stage_c(0)
            stage_b(2)
            stage_c(1)
            stage_b(3)
            stage_c(2)
            stage_c(3)

    nc.compile()
    return nc


def _make_in_maps(x, Wq, Wk, Wv, Wproj):
    bf = ml_dtypes.bfloat16
    xT = np.ascontiguousarray(x[0].T).astype(bf)  # [D, T]

    # rotary tables, rearranged to [tp, tb, 32] and flattened
    inv = 1.0 / (10000.0 ** (np.arange(0, HD, 2, dtype=np.float32) / HD))
    fr = np.outer(np.arange(T, dtype=np.float32), inv)  # [T, 32]
    cos = np.cos(fr).reshape(TB, 128, 32).transpose(1, 0, 2).reshape(128, -1)
    sin = np.sin(fr).reshape(TB, 128, 32).transpose(1, 0, 2).reshape(128, -1)
    cos, sin = cos.astype(bf), sin.astype(bf)
    diag = np.triu(np.ones((128, 128), np.float32)).astype(bf)

    in_maps = []
    for h in range(NCORES):
        wqk = np.concatenate(
            [
                Wq[h * 64 : h * 64 + 64],
                Wq[512 + h * 64 : 512 + h * 64 + 64],
                Wk[h * 64 : h * 64 + 64],
                Wk[512 + h * 64 : 512 + h * 64 + 64],
                Wv[h * 128 : h * 128 + 128],
            ],
            axis=0,
        ).T  # [D, 384]
        # wpp[j, i] = Wproj[i, h*128+j] -- lhsT chunks for the partial proj
        wpp = Wproj[:, h * 128 : (h + 1) * 128].T  # [128 j, 1024 i]
        in_maps.append(
            {
                "xT": xT,
                "wqkv": np.ascontiguousarray(wqk).astype(bf),
                "wpp": np.ascontiguousarray(wpp).astype(bf),
                "cos": cos,
                "sin": sin,
                "diag": diag,
            }
        )
    return in_maps


def _get_program(lam: float):
    key = round(lam, 10)
    if key not in _CACHE:
        _CACHE[key] = _build_program(lam)
    return _CACHE[key]


def kernel(x, Wq, Wk, Wv, Wproj, lambda_q1, lambda_k1, lambda_q2, lambda_k2):
    x = np.asarray(x, np.float32)
    Wq, Wk = np.asarray(Wq, np.float32), np.asarray(Wk, np.float32)
    Wv, Wproj = np.asarray(Wv, np.float32), np.asarray(Wproj, np.float32)

    lam1 = float(np.exp(np.sum(np.asarray(lambda_q1) * np.asarray(lambda_k1))))
    lam2 = float(np.exp(np.sum(np.asarray(lambda_q2) * np.asarray(lambda_k2))))
    lam = lam1 - lam2 + LAMBDA_INIT

    in_maps = _make_in_maps(x, Wq, Wk, Wv, Wproj)
    nc = _get_program(lam)

    res = run_bass_kernel_spmd(nc, in_maps, list(range(NCORES)))
    # unshard: row-parallel c_proj -> sum the 8 bf16 partial products in f32
    acc = res.results[0]["outTp"].astype(np.float32)
    for h in range(1, NCORES):
        acc += res.results[h]["outTp"].astype(np.float32)
    return np.ascontiguousarray(acc.T).reshape(1, T, D)


if __name__ == "__main__":
    rng = np.random.default_rng(0)
    ins = {
        "x": rng.standard_normal((1, T, D), np.float32),
        "Wq": (rng.standard_normal((D, D)) * 0.02).astype(np.float32),
        "Wk": (rng.standard_normal((D, D)) * 0.02).astype(np.float32),
        "Wv": (rng.standard_normal((D, D)) * 0.02).astype(np.float32),
        "Wproj": (rng.standard_normal((D, D)) * 0.02).astype(np.float32),
        "lambda_q1": (rng.standard_normal(32) * 0.1).astype(np.float32),
        "lambda_k1": (rng.standard_normal(32) * 0.1).astype(np.float32),
        "lambda_q2": (rng.standard_normal(32) * 0.1).astype(np.float32),
        "lambda_k2": (rng.standard_normal(32) * 0.1).astype(np.float32),
    }
    y = kernel(**ins)
    print("kernel output", y.shape, y.dtype, float(np.abs(y).mean()))
